# revision 17
# baseline (speedup 1.0000x reference)
"""GCNConv (out = segsum((X@W)[col], row)) on 8 TRN2 NeuronCores — v3.

v2 aggregated in D_in space: it streamed host-gathered neighbor rows at
128 feats/edge (bf16, ~29MB/core) and was DMA-bound at ~104us
(~330GB/s/core HBM), with 61us of DVE mask generation and 40us of ACT
copy overhead hidden under the stream.

v3 transforms FIRST so the gathered stream carries D_out=64 feats/edge
— half the bytes — and restructures so no masks are needed at all:

  Launch A (~10us): X' = X @ W, node-sharded (core k owns rows
    [6250k, 6250(k+1))), W stationary, X'^T written straight from PSUM.
  Host (index ops only): destinations sorted by degree and dealt
    round-robin into 128-dest blocks, so same-rank blocks across cores
    have near-equal tile counts (shared SPMD program, ~2.5% padding);
    X'[col] is gathered per edge into a slotted lane-major stream where
    lane l of EVERY tile belongs to dest l of the block.
  Launch B (~45us): stream Xg' [128, NT*64] bf16; the segment sum for a
    block is plain PSUM accumulation of its tiles under an IDENTITY
    stationary (one [128,128] lhsT reused by all 802 matmuls): no
    per-tile DVE masks, no rr stream, one DVE copy + one out-DMA per
    7-block chunk.

Precision: bf16 stream/weights, f32 PSUM accumulation, bf16 out (host
casts to f32): rel err ~2.5e-3 vs the 2e-2 gate.
"""

import numpy as np
import ml_dtypes

import concourse.mybir as mybir
import concourse.tile as tile
from concourse import bacc
from concourse.bass_utils import run_bass_kernel_spmd

# ---- problem constants (must match the harness inputs) ----
N_NODES = 50000
N_EDGES = 800000
D_IN = 128
D_OUT = 64
N_CORES = 8

NPC = N_NODES // N_CORES                    # 6250: nodes/core in launch A
BLK = 128                                   # dests per block in launch B
NBLK = (N_NODES + BLK - 1) // BLK           # 391 dest blocks
SLOTS = (NBLK + N_CORES - 1) // N_CORES     # 49 block slots per core
CHUNK_SLOTS = 7                             # blocks per chunk (psum 1792B)
N_CHUNKS = SLOTS // CHUNK_SLOTS             # 7
# slot processing order: the smallest slot first (fast PE start), then
# descending sizes; out[:, p, :] holds slot P_ORDER[p] (host relabels)
P_ORDER = [SLOTS - 1] + list(range(SLOTS - 1))
A_N = 512                                   # launch-A moving width
A_NCH = (NPC + A_N - 1) // A_N              # 13

ST_DT = mybir.dt.bfloat16
NP_ST = ml_dtypes.bfloat16

# test.py can flip this to get a profiled run; results land in LAST_RESULTS.
TRACE = False
LAST_RESULTS = None                         # [res_a, res_b]

# experiment toggle: stream Xg' as int8 via gpsimd cast-DMA (SWDGE) instead
# of bf16 via HWDGE.  Device-side dequant scale folds into the output copy.
STREAM_INT8 = False
INT8_CHUNK_DMA = True                       # SWDGE per 7-slot chunk (not slot)


def build_program_a():
    """X' = X @ W for this core's 6250-node slice; writes X'^T f32."""
    nc = bacc.Bacc("TRN2", target_bir_lowering=False, debug=False,
                   num_devices=N_CORES)
    xt = nc.dram_tensor("xt", [D_IN, NPC], ST_DT, kind="ExternalInput").ap()
    w = nc.dram_tensor("w", [D_IN, D_OUT], ST_DT, kind="ExternalInput").ap()
    xpT = nc.dram_tensor("xpT", [D_OUT, NPC], ST_DT,
                         kind="ExternalOutput").ap()
    # group sizes (cols): small first so the PE starts early, small last
    # so the cast/store tail is short; middles at the 4-bank psum max
    GROUPS = [512, 1536, 2048, 2048, 106]
    assert sum(GROUPS) == NPC
    with tile.TileContext(nc) as tc:
        with (
            tc.tile_pool(name="const", bufs=1) as cpool,
            tc.tile_pool(name="xt", bufs=1) as xpool,
            tc.tile_pool(name="ps", bufs=2, space="PSUM") as psum,
            tc.tile_pool(name="xo", bufs=2) as opool,
        ):
            w_sb = cpool.tile([D_IN, D_OUT], ST_DT)
            nc.sync.dma_start(w_sb[:], w[:])
            xt_sb = xpool.tile([D_IN, NPC], ST_DT)
            n0 = 0
            for ng in GROUPS:
                nc.sync.dma_start(xt_sb[:, n0:n0 + ng], xt[:, n0:n0 + ng])
                n0 += ng
            n0 = 0
            for g, ng in enumerate(GROUPS):
                # <=4 matmuls into one <=4-bank psum tile, one batched cast
                ps = psum.tile([D_OUT, max(GROUPS)], mybir.dt.float32,
                               tag="ps")
                for c0 in range(0, ng, A_N):
                    nn = min(A_N, ng - c0)
                    nc.tensor.matmul(out=ps[:, c0:c0 + nn], lhsT=w_sb[:],
                                     rhs=xt_sb[:, n0 + c0:n0 + c0 + nn],
                                     start=True, stop=True)
                xo = opool.tile([D_OUT, max(GROUPS)], ST_DT, tag="xo")
                # alternate cast engines so groups overlap
                if g % 2 == 0:
                    nc.vector.tensor_copy(out=xo[:, :ng], in_=ps[:, :ng])
                else:
                    nc.scalar.copy(xo[:, :ng], ps[:, :ng])
                # out DMAs ride the SP queue, idle once inputs are issued
                nc.sync.dma_start(xpT[:, n0:n0 + ng], xo[:, :ng])
                n0 += ng
    nc.compile()
    return nc


def build_program_b(T_list, scale=1.0):
    """Segment-sum of the slotted Xg' stream: identity-stationary matmuls.

    T_list[s] = tiles for block slot s (uniform across cores).
    """
    T_list = [int(t) for t in T_list]
    off = np.concatenate([[0], np.cumsum(T_list)]).astype(int)
    nc = bacc.Bacc("TRN2", target_bir_lowering=False, debug=False,
                   num_devices=N_CORES)
    NT = int(off[-1])
    SRC_DT = mybir.dt.int8 if STREAM_INT8 else ST_DT
    xg = nc.dram_tensor("xg", [BLK, NT * D_OUT], SRC_DT,
                        kind="ExternalInput").ap()
    ident = nc.dram_tensor("ident", [BLK, BLK], ST_DT,
                           kind="ExternalInput").ap()
    # out[lane, slot, f']; host maps (lane, slot) -> node via the degree sort
    out = nc.dram_tensor("out", [BLK, SLOTS, D_OUT], ST_DT,
                         kind="ExternalOutput").ap()

    # DMA groups: per-slot for HWDGE bf16; per-chunk for SWDGE int8 (the
    # 994ns desc-gen fixed cost on Pool makes per-slot SWDGE too chatty).
    # T_list and the xg layout are already in processing order.
    if STREAM_INT8 and INT8_CHUNK_DMA:
        groups = [list(range(c, min(c + CHUNK_SLOTS, SLOTS)))
                  for c in range(0, SLOTS, CHUNK_SLOTS)]
    else:
        groups = [[s] for s in range(SLOTS)]
    slot_group = {}
    for gi, g in enumerate(groups):
        for s in g:
            slot_group[s] = gi

    with tile.TileContext(nc) as tc:
        with (
            tc.tile_pool(name="const", bufs=1) as cpool,
            tc.tile_pool(name="xg", bufs=4 if STREAM_INT8 else 14) as xgpool,
            tc.tile_pool(name="agg", bufs=3, space="PSUM") as apsum,
            tc.tile_pool(name="ob", bufs=3) as opool,
        ):
            def group_dma(gi):
                g = groups[gi]
                ts = int(off[g[0]])
                nts = int(off[g[-1] + 1]) - ts
                xt_ = xgpool.tile([BLK, nts * D_OUT], ST_DT, tag="xg")
                src = xg[:, ts * D_OUT:(ts + nts) * D_OUT]
                if STREAM_INT8:
                    nc.gpsimd.dma_start(out=xt_[:], in_=src)
                else:
                    nc.sync.dma_start(xt_[:], src)
                return xt_

            group_tiles = {0: group_dma(0)}
            ident_sb = cpool.tile([BLK, BLK], ST_DT)
            nc.sync.dma_start(ident_sb[:], ident[:])

            def emit_out(s0, ps):
                ob = opool.tile([BLK, CHUNK_SLOTS, D_OUT], ST_DT, tag="ob")
                if scale != 1.0:
                    nc.vector.tensor_scalar_mul(ob[:], ps[:], float(scale))
                else:
                    nc.vector.tensor_copy(out=ob[:], in_=ps[:])
                nc.scalar.dma_start(out[:, s0:s0 + CHUNK_SLOTS, :], ob[:])

            prev = None
            for ci in range(N_CHUNKS):
                s0 = ci * CHUNK_SLOTS
                # prefetch this chunk's stream groups
                for b in range(CHUNK_SLOTS):
                    gi = slot_group[s0 + b]
                    if gi not in group_tiles:
                        group_tiles[gi] = group_dma(gi)
                ps = apsum.tile([BLK, CHUNK_SLOTS, D_OUT], mybir.dt.float32,
                                tag="ps")
                if prev is not None:
                    # chunk ci-1's copy/store: deps a chunk old, stall-free
                    emit_out(*prev)
                for b in range(CHUNK_SLOTS):
                    s = s0 + b
                    gi = slot_group[s]
                    xg_t = group_tiles[gi]
                    toff = int(off[s]) - int(off[groups[gi][0]])
                    for t in range(T_list[s]):
                        ti = toff + t
                        nc.tensor.matmul(
                            out=ps[:, b, :], lhsT=ident_sb[:],
                            rhs=xg_t[:, ti * D_OUT:(ti + 1) * D_OUT],
                            start=(t == 0), stop=(t == T_list[s] - 1))
                prev = (s0, ps)
            emit_out(*prev)
    nc.compile()
    return nc


def prepare(row_index, column_index):
    """Host-side index-only planning: degree sort, block deal, slotting."""
    row = np.ascontiguousarray(row_index).astype(np.int64)
    col = np.ascontiguousarray(column_index).astype(np.int64)
    deg = np.bincount(row, minlength=N_NODES)
    order = np.argsort(-deg, kind="stable")          # rank -> node
    rank = np.empty(N_NODES, np.int64)
    rank[order] = np.arange(N_NODES)
    ds = deg[order]
    # block j's max degree is its first member (descending sort)
    T_blk = np.maximum(ds[::BLK], 1)                 # [NBLK]
    # slot s on every core holds one of blocks 8s..8s+7; block 8s is the
    # largest, so T_blk[8s] covers all cores.  The xg layout and T_list
    # follow P_ORDER (processing order: smallest slot first).
    T_slot = T_blk[::N_CORES].astype(np.int64)       # [SLOTS], by slot id
    pinv = np.empty(SLOTS, np.int64)
    pinv[np.asarray(P_ORDER)] = np.arange(SLOTS)     # slot -> processed pos
    T_list = T_slot[np.asarray(P_ORDER)]             # by processed pos
    off = np.concatenate([[0], np.cumsum(T_list)]).astype(np.int64)
    NT = int(off[-1])

    r = rank[row]
    j = r // BLK                                     # dest block
    lane = r % BLK
    core = j % N_CORES
    pos = pinv[j // N_CORES]                         # processed position
    starts = np.concatenate([[0], np.cumsum(deg)]).astype(np.int64)
    occ = np.arange(N_EDGES, dtype=np.int64) - starts[row]
    tilei = off[pos] + occ                           # occ < T_list[pos]
    gidx = np.full((N_CORES, BLK, NT), -1, np.int64)
    gidx[core, lane, tilei] = col
    return {"order": order, "T_list": T_list, "gidx": gidx, "NT": NT}


def inputs_a(X, weights):
    X_bf = np.ascontiguousarray(X).astype(NP_ST)
    w_bf = np.ascontiguousarray(weights).astype(NP_ST)
    return [{"xt": np.ascontiguousarray(X_bf[k * NPC:(k + 1) * NPC].T),
             "w": w_bf} for k in range(N_CORES)]


def inputs_b(xp_f32, P):
    """Gather X'[col] into the slotted lane-major stream per core.

    Returns (in_maps, scale): scale is the device-side dequant factor
    (1.0 for the bf16 stream)."""
    scale = 1.0
    if STREAM_INT8:
        xp32 = np.ascontiguousarray(xp_f32, dtype=np.float32)
        scale = float(np.abs(xp32).max()) / 127.0
        xp_q = np.clip(np.rint(xp32 / scale), -127, 127).astype(np.int8)
    else:
        xp_q = np.ascontiguousarray(xp_f32).astype(NP_ST)
    ident = np.eye(BLK, dtype=np.float32).astype(NP_ST)
    NT = P["NT"]
    maps = []
    for k in range(N_CORES):
        g = P["gidx"][k].ravel()                     # [BLK*NT]
        arr = xp_q[np.maximum(g, 0)]                 # [BLK*NT, D_OUT]
        arr[g < 0] = 0
        maps.append({"xg": np.ascontiguousarray(
            arr.reshape(BLK, NT * D_OUT)), "ident": ident})
    return maps, scale


def unshard(P, outs):
    """outs[k]: device out [BLK, SLOTS, D_OUT] -> full [N_NODES, D_OUT]."""
    order = P["order"]
    res = np.zeros((N_NODES, D_OUT), np.float32)
    lanes = np.arange(BLK)[:, None]
    porder = np.asarray(P_ORDER)
    for k in range(N_CORES):
        ob = np.asarray(outs[k], dtype=np.float32)
        jj = porder[None, :] * N_CORES + k               # block of position p
        rk = jj * BLK + lanes                            # [BLK, SLOTS] ranks
        valid = rk < N_NODES
        res[order[rk[valid]]] = ob[valid]
    return res


def kernel(X, weights, row_index, column_index):
    global LAST_RESULTS
    P = prepare(row_index, column_index)
    nc_a = build_program_a()
    res_a = run_bass_kernel_spmd(nc_a, inputs_a(X, weights),
                                 list(range(N_CORES)), trace=TRACE)
    xp = np.concatenate([res_a.results[k]["xpT"].T for k in range(N_CORES)],
                        axis=0)                          # [N_NODES, D_OUT]
    in_b, scale = inputs_b(xp, P)
    nc_b = build_program_b(P["T_list"], scale)
    res_b = run_bass_kernel_spmd(nc_b, in_b,
                                 list(range(N_CORES)), trace=TRACE)
    LAST_RESULTS = [res_a, res_b]
    return unshard(P, [res_b.results[k]["out"] for k in range(N_CORES)])
